# revision 4
# baseline (speedup 1.0000x reference)
"""Trainium2 Bass kernel for the Bayesian SNN problem — v2.

vs baseline:
- GEMM2 runs in a SINGLE fp16 pass (w2 fp16, spikes exact in bf16) instead
  of two bf16 hi/lo passes. (fp32 G1 was tried: HW fp32 matmul = 4 cyc/row,
  worse than the 3-pass bf16 split, so G1 keeps the baseline scheme.)
- The two layers are interleaved in ONE phase: as each hidden tile's spikes
  come out of the LIF scan they are immediately contracted with the matching
  w2 row-block (PSUM-accumulated in quads of 4 hidden tiles) into SBUF cur2
  accumulators. This kills the spike DRAM round-trip, the phase-A/B boundary
  stall, and the w2 hi/lo generation work.

Per-core layout: hidden on partitions for GEMM1 (cur1[h, t*256+b]),
batch rows on partitions for GEMM2 output (cur2[b128, dout]).
"""

import numpy as np

import concourse.bass as bass
import concourse.tile as tile
from concourse import bacc, mybir
from concourse.bass_utils import run_bass_kernel_spmd

F32 = mybir.dt.float32
BF16 = mybir.dt.bfloat16
FP16 = mybir.dt.float16
ALU = mybir.AluOpType
ACTF = mybir.ActivationFunctionType

P = 128
B, T, DIN, DH, DOUT = 2048, 5, 2048, 4096, 1024
NCORES = 8
BC = B // NCORES            # 256 batch rows per core
TB = T * BC                 # 1280 batched-time cols per core
KO1 = DIN // P              # 16 contraction tiles, layer 1
NT = DH // P                # 32 hidden tiles
QUAD = 4                    # hidden tiles accumulated per PSUM group in GEMM2
NBLK = TB // P              # 10 (t, h128) column blocks
BETA = 0.95
THRESH = 1.0

# GEMM1 free-dim chunks over TB (PSUM bank = 512 fp32)
G1_CHUNKS = ((0, 512), (512, 512), (1024, 256))
# t -> (chunk_idx, offset inside chunk)
T_LOC = {0: (0, 0), 1: (0, 256), 2: (1, 0), 3: (1, 256), 4: (2, 0)}


def _build_nc():
    nc = bacc.Bacc(
        "TRN2",
        target_bir_lowering=False,
        debug=False,
        num_devices=NCORES,
    )

    xt = nc.dram_tensor("xt", [DIN, TB], F32, kind="ExternalInput").ap()
    w1_mu = nc.dram_tensor("w1_mu", [DIN, DH], F32, kind="ExternalInput").ap()
    w1_lv = nc.dram_tensor("w1_logvar", [DIN, DH], F32, kind="ExternalInput").ap()
    eps1 = nc.dram_tensor("eps1", [DIN, DH], F32, kind="ExternalInput").ap()
    w2_mu = nc.dram_tensor("w2_mu", [DH, DOUT], F32, kind="ExternalInput").ap()
    w2_lv = nc.dram_tensor("w2_logvar", [DH, DOUT], F32, kind="ExternalInput").ap()
    eps2 = nc.dram_tensor("eps2", [DH, DOUT], F32, kind="ExternalInput").ap()
    out = nc.dram_tensor("out", [BC, DOUT], F32, kind="ExternalOutput").ap()

    with tile.TileContext(nc) as tc:
        with (
            tc.tile_pool(name="xres", bufs=1) as xp,
            tc.tile_pool(name="xstg", bufs=2) as xsp,
            tc.tile_pool(name="w1hl", bufs=2) as wp1,
            tc.tile_pool(name="w1st", bufs=1) as stp,
            tc.tile_pool(name="spk", bufs=6) as sp,
            tc.tile_pool(name="mem1", bufs=2) as mp,
            tc.tile_pool(name="w2st", bufs=1) as w2s,
            tc.tile_pool(name="w2f", bufs=6) as w2f,
            tc.tile_pool(name="acc", bufs=1) as accp,
            tc.tile_pool(name="lif2", bufs=1) as l2p,
            tc.tile_pool(name="g1ps", bufs=4, space="PSUM") as g1p,
            tc.tile_pool(name="g2ps", bufs=4, space="PSUM") as g2p,
        ):
            ACC = accp.tile([P, NBLK, DOUT], F32)

            HKO = KO1 // 2

            def emit_w1gen_dma(n):
                """Issue DMAs for w1 column-block n in two row-halves
                (sync queue). Returns the half tiles."""
                ncols = slice(n * P, (n + 1) * P)
                halves = []
                for q in range(2):
                    rows = slice(q * HKO * P, (q + 1) * HKO * P)
                    mut = stp.tile([P, HKO, P], F32, tag="mu1",
                                   name=f"mu1_{q}")
                    lvt = stp.tile([P, HKO, P], F32, tag="lv1",
                                   name=f"lv1_{q}")
                    ept = stp.tile([P, HKO, P], F32, tag="ep1",
                                   name=f"ep1_{q}")
                    nc.sync.dma_start(
                        mut,
                        w1_mu[rows, ncols].rearrange("(o p) n -> p o n", p=P),
                    )
                    nc.sync.dma_start(
                        lvt,
                        w1_lv[rows, ncols].rearrange("(o p) n -> p o n", p=P),
                    )
                    nc.scalar.dma_start(
                        ept,
                        eps1[rows, ncols].rearrange("(o p) n -> p o n", p=P),
                    )
                    halves.append((mut, lvt, ept))
                return halves

            def emit_w1gen_compute(halves, quarters=False):
                """w1 = mu + eps*exp(0.5*lv) fp32, then split bf16 hi/lo.
                quarters=True emits the ops in quarter slices so the first
                k-tiles' weights are ready sooner (startup only)."""
                w1h = wp1.tile([P, KO1, P], BF16, tag="w1h")
                w1l = wp1.tile([P, KO1, P], BF16, tag="w1l")
                nsub = 2 if quarters else 1
                for q, (mut, lvt, ept) in enumerate(halves):
                    for u in range(nsub):
                        usl = slice(u * HKO // nsub, (u + 1) * HKO // nsub)
                        lvf = lvt[:, usl, :].rearrange("p a b -> p (a b)")
                        epf = ept[:, usl, :].rearrange("p a b -> p (a b)")
                        muf = mut[:, usl, :].rearrange("p a b -> p (a b)")
                        nc.scalar.activation(lvf, lvf, ACTF.Exp, scale=0.5)
                        nc.vector.scalar_tensor_tensor(
                            epf, epf, 1.0, lvf, op0=ALU.bypass, op1=ALU.mult
                        )
                        nc.vector.tensor_tensor(muf, muf, epf, op=ALU.add)
                        h0 = q * HKO + u * HKO // nsub
                        hsl = slice(h0, h0 + HKO // nsub)
                        w1hf = w1h[:, hsl, :].rearrange("p a b -> p (a b)")
                        w1lf = w1l[:, hsl, :].rearrange("p a b -> p (a b)")
                        nc.vector.tensor_copy(w1hf, muf)
                        nc.vector.scalar_tensor_tensor(
                            w1lf, muf, 1.0, w1hf,
                            op0=ALU.bypass, op1=ALU.subtract,
                        )
                return w1h, w1l

            def emit_w2gen(n):
                """w2 row-block n -> fp16 tile (scalar queue DMAs)."""
                orows = slice(n * P, (n + 1) * P)
                wf = w2f.tile([P, DOUT], FP16, tag="w2f")
                for hf in range(2):
                    csl = slice(hf * 512, (hf + 1) * 512)
                    m2 = w2s.tile([P, 512], F32, tag="mu2")
                    l2 = w2s.tile([P, 512], F32, tag="lv2")
                    e2 = w2s.tile([P, 512], F32, tag="ep2")
                    nc.gpsimd.dma_start(m2, w2_mu[orows, csl])
                    nc.gpsimd.dma_start(l2, w2_lv[orows, csl])
                    nc.gpsimd.dma_start(e2, eps2[orows, csl])
                    nc.scalar.activation(l2, l2, ACTF.Exp, scale=0.5)
                    nc.gpsimd.tensor_tensor(e2, e2, l2, op=ALU.mult)
                    nc.gpsimd.tensor_tensor(m2, m2, e2, op=ALU.add)
                    nc.scalar.activation(wf[:, csl], m2, ACTF.Copy)
                return wf

            # ---- PE warm-up: dummy matmuls through the startup DMA window
            # keep the HAM clock-gate at full rate for the first real tiles ----
            warm = accp.tile([P, 512], BF16, tag="warm", name="warm")
            nc.gpsimd.memset(warm, 0.0)
            wps = g1p.tile([P, 512], F32, tag="g1", name="warmps")
            for i in range(40):
                nc.tensor.matmul(
                    wps, warm[:, :128], warm,
                    start=(i == 0), stop=(i == 39),
                )

            # ---- startup: w1 block 0 DMAs, then x residency (staged fp32
            # round-robin over the three DMA queues, split to bf16 hi/lo) ----
            w1_pend = emit_w1gen_dma(0)
            w1cur = emit_w1gen_compute(w1_pend, quarters=True)
            XH = [
                xp.tile([P, TB], BF16, tag=f"xh{o}", name=f"xh{o}")
                for o in range(KO1)
            ]
            XL = [
                xp.tile([P, TB], BF16, tag=f"xl{o}", name=f"xl{o}")
                for o in range(KO1)
            ]
            qs = [nc.gpsimd, nc.scalar, nc.sync]
            for o in range(KO1):
                xs = xsp.tile([P, TB], F32, tag="xstg")
                qs[o % 3].dma_start(xs, xt[o * P : (o + 1) * P, :])
                nc.scalar.activation(XH[o], xs, ACTF.Copy)
                nc.vector.scalar_tensor_tensor(
                    XL[o], xs, 1.0, XH[o], op0=ALU.bypass, op1=ALU.subtract
                )

            # LIF2 state: mem2 lives in-place in ACC's t=0 block
            l2st = {
                h: (
                    ACC[:, h, :],
                    l2p.tile([P, DOUT], BF16, tag=f"spk2_{h}", name=f"spk2_{h}"),
                    l2p.tile([P, DOUT], F32, tag=f"oacc_{h}", name=f"oacc_{h}"),
                )
                for h in range(2)
            }

            def emit_lif2_step(t):
                """One LIF2 timestep for both h-halves, interleaved on
                vector so the two chains hide each other's latency."""
                for h in range(2):
                    mem2, spk2, oacc = l2st[h]
                    cur = ACC[:, t * 2 + h, :]
                    if t == 0:
                        nc.vector.tensor_scalar(
                            oacc, mem2, THRESH, None, op0=ALU.is_gt
                        )
                        nc.scalar.activation(spk2, oacc, ACTF.Copy)
                    else:
                        nc.vector.scalar_tensor_tensor(
                            mem2, mem2, BETA, spk2,
                            op0=ALU.mult, op1=ALU.subtract,
                        )
                        nc.vector.scalar_tensor_tensor(
                            mem2, mem2, 1.0, cur, op0=ALU.bypass, op1=ALU.add
                        )
                        if t < T - 1:
                            nc.vector.tensor_scalar(
                                spk2, mem2, THRESH, None, op0=ALU.is_gt
                            )
                        nc.vector.scalar_tensor_tensor(
                            oacc, mem2, THRESH, oacc,
                            op0=ALU.is_gt, op1=ALU.add,
                        )

            spks = {}
            w2fs = {}
            for n in range(NT):
                # ---- prefetch w1 block n+1 DMAs (execute during G1(n)) ----
                if n + 1 < NT:
                    w1_pend = emit_w1gen_dma(n + 1)

                # ---- GEMM1 tile n: 3-pass bf16 hi/lo, chunk-exclusive
                # accumulation so each PSUM chunk completes and frees early;
                # the LIF scan pipelines chunk-by-chunk behind the PE.
                # Tile 0 runs k-outer instead so the PE starts as soon as
                # the first x k-slice lands (x DMAs are still streaming). ----
                w1h, w1l = w1cur
                spkt = sp.tile([P, TB], BF16, tag="spk")
                mem = mp.tile([P, BC], F32, tag="mem1")

                def lif_steps(ci, ps):
                    for t in range(T):
                        tci, toff = T_LOC[t]
                        if tci != ci:
                            continue
                        cur = ps[:, toff : toff + BC]
                        tsl = slice(t * BC, (t + 1) * BC)
                        if t == 0:
                            nc.scalar.activation(mem, cur, ACTF.Copy)
                        else:
                            psl = slice((t - 1) * BC, t * BC)
                            nc.vector.scalar_tensor_tensor(
                                mem, mem, BETA, spkt[:, psl],
                                op0=ALU.mult, op1=ALU.subtract,
                            )
                            nc.vector.scalar_tensor_tensor(
                                mem, mem, 1.0, cur, op0=ALU.bypass, op1=ALU.add
                            )
                        nc.vector.tensor_scalar(
                            spkt[:, tsl], mem, THRESH, None, op0=ALU.is_gt
                        )

                if n == 0:
                    pss = [g1p.tile([P, 512], F32, tag="g1", name=f"g1z{c}")
                           for c in range(3)]
                    for k in range(KO1):
                        for pi, (lt, rt) in enumerate(
                            ((w1h, XH), (w1h, XL), (w1l, XH))
                        ):
                            for ci, (c0, cw) in enumerate(G1_CHUNKS):
                                nc.tensor.matmul(
                                    pss[ci][:, :cw],
                                    lt[:, k, :],
                                    rt[k][:, c0 : c0 + cw],
                                    start=(pi == 0 and k == 0),
                                    stop=(pi == 2 and k == KO1 - 1),
                                )
                    if n + 1 < NT:
                        w1next = emit_w1gen_compute(w1_pend)
                    for ci in range(3):
                        lif_steps(ci, pss[ci])
                else:
                    pss2 = []
                    for ci, (c0, cw) in enumerate(G1_CHUNKS):
                        ps = g1p.tile([P, 512], F32, tag="g1")
                        pss2.append(ps)
                        for pi, (lt, rt) in enumerate(
                            ((w1h, XH), (w1h, XL), (w1l, XH))
                        ):
                            for k in range(KO1):
                                nc.tensor.matmul(
                                    ps[:, :cw],
                                    lt[:, k, :],
                                    rt[k][:, c0 : c0 + cw],
                                    start=(pi == 0 and k == 0),
                                    stop=(pi == 2 and k == KO1 - 1),
                                )
                    # w1(n+1) vector ops BEFORE the LIF ops in the vector
                    # FIFO: they are DMA-ready mid-tile, so G1(n+1) never
                    # waits on weight generation behind the LIF chain
                    if n + 1 < NT:
                        w1next = emit_w1gen_compute(w1_pend)
                    for ci in range(3):
                        lif_steps(ci, pss2[ci])
                spks[n] = spkt

                # ---- w2 block n (fp16) ----
                w2fs[n] = emit_w2gen(n)

                # ---- GEMM2, one tile delayed: members' LIF scans are
                # long done, so these matmuls have no pending deps and the
                # PE never stalls at the quad boundary ----
                if n % QUAD == 0 and n >= QUAD:
                    first_quad = n == QUAD
                    last_quad = False
                    members = list(range(n - QUAD, n))
                    for blk in range(NBLK):
                        bsl = slice(blk * P, (blk + 1) * P)
                        # two open accumulation groups (one per 512-chunk)
                        # so each spk slice is loaded as weights only once
                        psA = g2p.tile([P, 512], F32, tag="g2")
                        psB = g2p.tile([P, 512], F32, tag="g2")
                        for j, m in enumerate(members):
                            for ps2, c2 in ((psA, 0), (psB, 512)):
                                nc.tensor.matmul(
                                    ps2,
                                    spks[m][:, bsl],
                                    w2fs[m][:, c2 : c2 + 512],
                                    start=(j == 0),
                                    stop=(j == QUAD - 1),
                                )
                        for ps2, c2 in ((psA, 0), (psB, 512)):
                            dst = ACC[:, blk, c2 : c2 + 512]
                            if first_quad:
                                if blk % 2 == 0:
                                    nc.vector.tensor_copy(dst, ps2)
                                else:
                                    nc.scalar.activation(dst, ps2, ACTF.Copy)
                            else:
                                nc.vector.scalar_tensor_tensor(
                                    dst, dst, 1.0, ps2,
                                    op0=ALU.bypass, op1=ALU.add,
                                )
                        if last_quad and blk % 2 == 1:
                            # ACC blocks 2t,2t+1 final: run LIF2 step t now
                            # so its chain hides under the remaining drains
                            emit_lif2_step(blk // 2)
                    for m in members:
                        spks.pop(m)
                        w2fs.pop(m)

                if n + 1 < NT:
                    w1cur = w1next


            # ---- final quad {NT-QUAD..NT-1} + interleaved LIF2 ----
            first_quad = False
            last_quad = True
            n = NT
            members = list(range(NT - QUAD, NT))
            for blk in range(NBLK):
                bsl = slice(blk * P, (blk + 1) * P)
                psA = g2p.tile([P, 512], F32, tag="g2", name="g2fA")
                psB = g2p.tile([P, 512], F32, tag="g2", name="g2fB")
                for j, m in enumerate(members):
                    for ps2, c2 in ((psA, 0), (psB, 512)):
                        nc.tensor.matmul(
                            ps2,
                            spks[m][:, bsl],
                            w2fs[m][:, c2 : c2 + 512],
                            start=(j == 0),
                            stop=(j == QUAD - 1),
                        )
                for ps2, c2 in ((psA, 0), (psB, 512)):
                    dst = ACC[:, blk, c2 : c2 + 512]
                    nc.vector.scalar_tensor_tensor(
                        dst, dst, 1.0, ps2, op0=ALU.bypass, op1=ALU.add
                    )
                if blk % 2 == 1:
                    emit_lif2_step(blk // 2)

            # ---- output (LIF2 steps were emitted inside the last quad) ----
            for h in range(2):
                nc.scalar.dma_start(out[h * P : (h + 1) * P, :], l2st[h][2])

    nc.compile()
    return nc


_NC_CACHE = None


def _get_nc():
    global _NC_CACHE
    if _NC_CACHE is None:
        _NC_CACHE = _build_nc()
    return _NC_CACHE


def _make_in_maps(inputs):
    x = np.ascontiguousarray(inputs["x"], dtype=np.float32)
    shared = {
        name: np.ascontiguousarray(inputs[name], dtype=np.float32)
        for name in ("w1_mu", "w1_logvar", "eps1", "w2_mu", "w2_logvar", "eps2")
    }
    in_maps = []
    for c in range(NCORES):
        xc = x[c * BC : (c + 1) * BC]          # [BC, T, DIN]
        xtc = np.ascontiguousarray(xc.transpose(2, 1, 0)).reshape(DIN, TB)
        in_maps.append({"xt": xtc, **shared})
    return in_maps


def _run(inputs, trace=False, **kwargs):
    nc = _get_nc()
    in_maps = _make_in_maps(inputs)
    res = run_bass_kernel_spmd(
        nc, in_maps, core_ids=list(range(NCORES)), trace=trace, **kwargs
    )
    outs = [np.asarray(res.results[c]["out"]) for c in range(NCORES)]
    full = np.concatenate(outs, axis=0).astype(np.float32)
    return full, res


def kernel(**inputs):
    full, _ = _run(inputs, trace=False)
    return full
